# revision 13
# baseline (speedup 1.0000x reference)
"""Bahdanau additive attention on 8 TRN2 NeuronCores (Bass/Tile).

Reference computation (per batch b):
    proj_f = features @ W1 + b1                         [T, U]
    proj_h = hidden @ W2 + b2                           [U]
    score  = tanh(proj_f + proj_h)                      [T, U]
    logits = score @ V + bV                             [T, 1]
    attn   = softmax(logits, axis=T)                    [T, 1]
    ctx    = sum_t attn[t] * features[t, :]             [D]
    returns (ctx [B, D], attn [B, T, 1])

Sharding: data-parallel over batch. B=32 across 8 cores -> 4 batches
per core; W1/W2/V replicated. No collectives.

Per-core layout:
  - The big matmul produces proj^T tiles [u=128, t=512]: W1 chunks
    [d=128, u=128] are the stationary operand, features^T [d=128,
    t=512] the moving operand (features fed pre-transposed per batch
    as fT [D, T]).  This makes proj_h a per-partition bias (fused into
    the tanh ACTIVATE, together with b1+b2) and makes the V
    contraction a partition-dim matmul.
  - Heavy matmuls run as float32r (full PE rate at N>=256), fp32 PSUM.
  - logits live as a [1, T] row per batch; softmax runs on one
    partition with fused exp+sum (ACT accum_out).  bV is dropped:
    softmax is shift-invariant, so bV cannot affect either output.
  - Attention weights are transposed to [t=128, 1] columns with tiny
    PE matmuls against [[1.0]] so the context matvec contracts t on
    the partition dim, streaming a second, natural-layout bf16 copy
    of features (fN).  bf16 halves both the DMA and the SBUF
    residency of that copy; context error stays ~1e-3.
  - Software pipelining: batch b's context phase is emitted in the
    middle of batch b+1's proj stream so the PE never waits on the
    softmax chain.
"""

import sys

sys.path.insert(0, "/opt/trn_rl_repo")

import numpy as np

B, T, D, U = 32, 2048, 1024, 1024
N_CORES = 8
B_LOC = B // N_CORES
P = 128
DC = D // P  # 8 d-chunks
UC = U // P  # 8 u-chunks
TCH = 512  # t-chunk (= one PSUM bank, = fp32 moving-operand max)
N_TC = T // TCH  # 4 macro steps per batch
N_CC = T // P  # 16 natural-layout feature chunks per batch


def build_nc():
    import concourse.tile as tile
    from concourse import bacc, mybir

    f32 = mybir.dt.float32
    f32r = mybir.dt.float32r
    bf16 = mybir.dt.bfloat16
    AF = mybir.ActivationFunctionType

    nc = bacc.Bacc("TRN2", target_bir_lowering=False, debug=False, num_devices=N_CORES)

    fT_ext = nc.declare_dram_parameter("fT", [B_LOC, D, T], bf16, isOutput=False)
    fN_ext = nc.declare_dram_parameter("fN", [B_LOC, T, D], bf16, isOutput=False)
    hT_ext = nc.declare_dram_parameter("hT", [D, B_LOC], bf16, isOutput=False)
    w1_ext = nc.declare_dram_parameter("w1", [D, U], bf16, isOutput=False)
    w2_ext = nc.declare_dram_parameter("w2", [D, U], bf16, isOutput=False)
    v_ext = nc.declare_dram_parameter("vv", [U, 1], bf16, isOutput=False)
    b12_ext = nc.declare_dram_parameter("b12", [U], f32, isOutput=False)
    ctx_ext = nc.declare_dram_parameter("ctx_out", [B_LOC, D], f32, isOutput=True)
    attn_ext = nc.declare_dram_parameter("attn_out", [B_LOC, T], f32, isOutput=True)

    fT_ap = fT_ext.ap().rearrange("b (dc p) t -> p b dc t", p=P)  # [128, 4, 8, T]
    fN_ap = fN_ext.ap().rearrange("b (cc p) d -> p b cc d", p=P)  # [128, 4, 16, D]

    with tile.TileContext(nc) as tc:
        with (
            tc.tile_pool(name="consts", bufs=1) as consts,
            tc.tile_pool(name="ft_pool", bufs=2) as ft_pool,
            tc.tile_pool(name="fn_pool", bufs=24) as fn_pool,
            tc.tile_pool(name="s_pool", bufs=3) as s_pool,
            tc.tile_pool(name="row_pool", bufs=2) as row_pool,
            tc.tile_pool(name="small", bufs=4) as small,
            tc.tile_pool(name="proj_ps", bufs=2, space="PSUM") as proj_ps,
            tc.tile_pool(name="logit_ps", bufs=2, space="PSUM") as logit_ps,
            tc.tile_pool(name="misc_ps", bufs=2, space="PSUM") as misc_ps,
        ):
            # ---- constants / weights ----
            v_sb = consts.tile([P, UC], bf16)
            nc.sync.dma_start(
                out=v_sb, in_=v_ext.ap().rearrange("(uc p) o -> p (uc o)", p=P)
            )
            b12_sb = consts.tile([P, UC], f32)
            nc.sync.dma_start(
                out=b12_sb, in_=b12_ext.ap().rearrange("(uc p) -> p uc", p=P)
            )
            ones11_bf = consts.tile([1, 1], bf16)
            nc.vector.memset(ones11_bf, 1.0)

            # ---- weight/feature DMA order tuned for fast pipeline start:
            # hT + w2/w1 slice 0 + first feature tile first, then alternating
            # w2/w1 u-slices.  The ph matmul groups are interleaved into the
            # first macro-step's proj slots (emit_ph_group below) so the
            # in-order PE stream never waits on the full W2 load.
            ph_sb = consts.tile([P, UC, B_LOC], f32)
            w1_sb = consts.tile([P, DC, U], bf16)
            w2_sb = consts.tile([P, DC, U], bf16)
            hT_sb = consts.tile([P, DC, B_LOC], bf16)
            nc.sync.dma_start(
                out=hT_sb, in_=hT_ext.ap().rearrange("(dc p) b -> p dc b", p=P)
            )
            w1_ap = w1_ext.ap().rearrange("(dc p) u -> p dc u", p=P)
            w2_ap = w2_ext.ap().rearrange("(dc p) u -> p dc u", p=P)
            nc.sync.dma_start(out=w2_sb[:, :, 0:P], in_=w2_ap[:, :, 0:P])
            nc.sync.dma_start(out=w1_sb[:, :, 0:P], in_=w1_ap[:, :, 0:P])
            ft0 = ft_pool.tile([P, DC, TCH], bf16, tag="ft", name="ft0")
            nc.sync.dma_start(out=ft0, in_=fT_ap[:, 0, :, 0:TCH])
            for uc in range(1, UC):
                nc.sync.dma_start(
                    out=w2_sb[:, :, uc * P : (uc + 1) * P],
                    in_=w2_ap[:, :, uc * P : (uc + 1) * P],
                )
                nc.sync.dma_start(
                    out=w1_sb[:, :, uc * P : (uc + 1) * P],
                    in_=w1_ap[:, :, uc * P : (uc + 1) * P],
                )

            # ---- pipelined main loop over (b, tc) macro-steps ----
            l_sb = {}  # b -> [1, T] logits row (exp'd in place)
            e16_sb = {}  # b -> [1, T] bf16 copy of exp'd logits
            rinv = {}
            fn_tiles = {}  # (b, cc) -> bf16 [128, D]
            s_tiles = {}  # (tc, uc) -> score tile
            logit_tiles = {}  # tc -> psum tile [1, TCH]
            ft_cur = [None]

            def emit_ph_group(uc):
                ph_ps = misc_ps.tile([P, B_LOC], f32, tag="misc", name="ph_ps")
                for dc in range(DC):
                    nc.tensor.matmul(
                        ph_ps,
                        lhsT=w2_sb[:, dc, uc * P : (uc + 1) * P],
                        rhs=hT_sb[:, dc, :],
                        start=(dc == 0),
                        stop=(dc == DC - 1),
                    )
                nc.vector.tensor_scalar_add(
                    out=ph_sb[:, uc, :], in0=ph_ps, scalar1=b12_sb[:, uc : uc + 1]
                )

            def emit_proj(step, uc):
                b, tc = step
                ps = proj_ps.tile([P, TCH], f32, tag="proj")
                for dc in range(DC):
                    nc.tensor.matmul(
                        ps,
                        lhsT=w1_sb[:, dc, uc * P : (uc + 1) * P],
                        rhs=ft_cur[0][:, dc, :],
                        start=(dc == 0),
                        stop=(dc == DC - 1),
                    )
                s = s_pool.tile([P, TCH], bf16, tag="s")
                nc.scalar.activation(
                    out=s, in_=ps, func=AF.Tanh, bias=ph_sb[:, uc, b : b + 1], scale=1.0
                )
                s_tiles[(tc, uc)] = s

            def emit_v(step, uc):
                b, tc = step
                s = s_tiles.pop((tc, uc))
                if uc == 0:
                    logit_tiles[tc] = logit_ps.tile([1, TCH], f32, tag="lg", name="lg")
                nc.tensor.matmul(
                    logit_tiles[tc],
                    lhsT=v_sb[:, uc : uc + 1],
                    rhs=s,
                    start=(uc == 0),
                    stop=(uc == UC - 1),
                )
                if uc == UC - 1:
                    off = tc * TCH
                    nc.vector.tensor_copy(
                        out=l_sb[b][0:1, off : off + TCH], in_=logit_tiles.pop(tc)
                    )

            def emit_softmax(b):
                esum = small.tile([1, 1], f32, tag="es")
                # exp in place over the logits row, free running sum.
                # No max-subtraction: logits here are O(+-4) (tanh in [-1,1],
                # V ~ N(0, 1/U)); softmax is shift-invariant and fp32 exp is
                # safe far beyond that range.
                nc.scalar.activation(
                    out=l_sb[b], in_=l_sb[b], func=AF.Exp, bias=0.0, accum_out=esum
                )
                r = small.tile([1, 1], f32, tag="ri")
                nc.vector.reciprocal(out=r, in_=esum)
                rinv[b] = r
                e16 = row_pool.tile([1, T], bf16, tag="e16")
                nc.vector.tensor_copy(out=e16, in_=l_sb[b])
                e16_sb[b] = e16
                a = row_pool.tile([1, T], f32, tag="a")
                nc.vector.tensor_scalar_mul(out=a, in0=l_sb.pop(b), scalar1=r)
                nc.sync.dma_start(out=attn_ext.ap()[b : b + 1, :], in_=a)

            def emit_ctx(b):
                # transpose exp'd logits [1, T] into bf16 columns [128, T/128];
                # the softmax normalization is folded into the ctx copy scale.
                aT = small.tile([P, N_CC], bf16, tag="aT")
                for i in range(N_CC):
                    tp = misc_ps.tile([P, 1], f32, tag="misc")
                    nc.tensor.matmul(
                        tp,
                        lhsT=e16_sb[b][0:1, i * P : (i + 1) * P],
                        rhs=ones11_bf,
                        start=True,
                        stop=True,
                    )
                    nc.vector.tensor_copy(out=aT[:, i : i + 1], in_=tp)
                ctx_row = small.tile([1, D], f32, tag="ctx")
                for dh in range(D // TCH):
                    cps = misc_ps.tile([1, TCH], f32, tag="misc")
                    for cc in range(N_CC):
                        nc.tensor.matmul(
                            cps,
                            lhsT=aT[:, cc : cc + 1],
                            rhs=fn_tiles[(b, cc)][:, dh * TCH : (dh + 1) * TCH],
                            start=(cc == 0),
                            stop=(cc == N_CC - 1),
                        )
                    nc.vector.tensor_scalar_mul(
                        out=ctx_row[0:1, dh * TCH : (dh + 1) * TCH],
                        in0=cps,
                        scalar1=rinv[b],
                    )
                nc.sync.dma_start(out=ctx_ext.ap()[b : b + 1, :], in_=ctx_row)
                for cc in range(N_CC):
                    fn_tiles.pop((b, cc))
                e16_sb.pop(b)
                rinv.pop(b)

            steps = [(b, tc) for b in range(B_LOC) for tc in range(N_TC)]
            pending = []  # [(step, uc)] whose V matmuls are pending (2-slot lag)
            for step in steps:
                b, tc = step
                if tc == 0:
                    l_sb[b] = row_pool.tile([1, T], f32, tag="l", name="lrow")
                if b == 0 and tc == 0:
                    ft = ft0
                else:
                    ft = ft_pool.tile([P, DC, TCH], bf16, tag="ft")
                    nc.sync.dma_start(
                        out=ft, in_=fT_ap[:, b, :, tc * TCH : (tc + 1) * TCH]
                    )
                # natural-layout bf16 feature chunks for next batch's ctx phase
                for cc in range(tc * N_CC // N_TC, (tc + 1) * N_CC // N_TC):
                    fn = fn_pool.tile([P, D], bf16, tag="fn", name="fn")
                    nc.sync.dma_start(out=fn, in_=fN_ap[:, b, cc, :])
                    fn_tiles[(b, cc)] = fn

                for uc in range(UC):
                    ft_cur[0] = ft
                    if b == 0 and tc == 0:
                        emit_ph_group(uc)
                    emit_proj(step, uc)
                    pending.append((step, uc))
                    if len(pending) > 2:
                        pstep, puc = pending.pop(0)
                        emit_v(pstep, puc)
                        if puc == UC - 1 and pstep[1] == N_TC - 1:
                            emit_softmax(pstep[0])
                    if uc == 4 and tc == 0 and b >= 1:
                        emit_ctx(b - 1)

            for pstep, puc in pending:
                emit_v(pstep, puc)
            emit_softmax(B_LOC - 1)
            emit_ctx(B_LOC - 1)

    nc.compile()
    return nc


_NC_CACHE = None


def _get_nc():
    global _NC_CACHE
    if _NC_CACHE is None:
        _NC_CACHE = build_nc()
    return _NC_CACHE


def make_in_maps(features, hidden, W1, b1, W2, b2, V, bV):
    import ml_dtypes

    features = np.asarray(features, dtype=np.float32)
    hidden = np.asarray(hidden, dtype=np.float32)
    W1 = np.ascontiguousarray(np.asarray(W1, dtype=np.float32))
    W2 = np.ascontiguousarray(np.asarray(W2, dtype=np.float32))
    V = np.ascontiguousarray(np.asarray(V, dtype=np.float32).reshape(U, 1))
    b12 = np.asarray(b1, dtype=np.float32) + np.asarray(b2, dtype=np.float32)
    # bV is dropped: softmax output is invariant to a constant logit shift.

    in_maps = []
    for c in range(N_CORES):
        sl = slice(c * B_LOC, (c + 1) * B_LOC)
        f = features[sl]
        in_maps.append(
            {
                "fT": np.ascontiguousarray(f.transpose(0, 2, 1).astype(ml_dtypes.bfloat16)),
                "fN": np.ascontiguousarray(f.astype(ml_dtypes.bfloat16)),
                "hT": np.ascontiguousarray(hidden[sl].T.astype(ml_dtypes.bfloat16)),
                "w1": W1.astype(ml_dtypes.bfloat16),
                "w2": W2.astype(ml_dtypes.bfloat16),
                "vv": V.astype(ml_dtypes.bfloat16),
                "b12": b12,
            }
        )
    return in_maps


def kernel(features, hidden, W1, b1, W2, b2, V, bV):
    from concourse.bass_utils import run_bass_kernel_spmd

    nc = _get_nc()
    in_maps = make_in_maps(features, hidden, W1, b1, W2, b2, V, bV)
    res = run_bass_kernel_spmd(nc, in_maps, core_ids=list(range(N_CORES)))
    ctx = np.concatenate([res.results[c]["ctx_out"] for c in range(N_CORES)], axis=0)
    attn = np.concatenate([res.results[c]["attn_out"] for c in range(N_CORES)], axis=0)
    return ctx, attn[:, :, None]


# revision 14
# speedup vs baseline: 1.1831x; 1.1831x over previous
"""Bahdanau additive attention on 8 TRN2 NeuronCores (Bass/Tile).

Reference computation (per batch b):
    proj_f = features @ W1 + b1                         [T, U]
    proj_h = hidden @ W2 + b2                           [U]
    score  = tanh(proj_f + proj_h)                      [T, U]
    logits = score @ V + bV                             [T, 1]
    attn   = softmax(logits, axis=T)                    [T, 1]
    ctx    = sum_t attn[t] * features[t, :]             [D]
    returns (ctx [B, D], attn [B, T, 1])

Sharding: data-parallel over batch. B=32 across 8 cores -> 4 batches
per core; W1/W2/V replicated. No collectives.

Per-core layout:
  - The big matmul produces proj^T tiles [u=128, t=512]: W1 chunks
    [d=128, u=128] are the stationary operand, features^T [d=128,
    t=512] the moving operand (features fed pre-transposed per batch
    as fT [D, T]).  This makes proj_h a per-partition bias (fused into
    the tanh ACTIVATE, together with b1+b2) and makes the V
    contraction a partition-dim matmul.
  - Heavy matmuls run as float32r (full PE rate at N>=256), fp32 PSUM.
  - logits live as a [1, T] row per batch; softmax runs on one
    partition with fused exp+sum (ACT accum_out).  bV is dropped:
    softmax is shift-invariant, so bV cannot affect either output.
  - Attention weights are transposed to [t=128, 1] columns with tiny
    PE matmuls against [[1.0]] so the context matvec contracts t on
    the partition dim, streaming a second, natural-layout bf16 copy
    of features (fN).  bf16 halves both the DMA and the SBUF
    residency of that copy; context error stays ~1e-3.
  - Software pipelining: batch b's context phase is emitted in the
    middle of batch b+1's proj stream so the PE never waits on the
    softmax chain.
"""

import sys

sys.path.insert(0, "/opt/trn_rl_repo")

import numpy as np

B, T, D, U = 32, 2048, 1024, 1024
N_CORES = 8
B_LOC = B // N_CORES
P = 128
DC = D // P  # 8 d-chunks
UC = U // P  # 8 u-chunks
TCH = 512  # t-chunk (= one PSUM bank, = fp32 moving-operand max)
N_TC = T // TCH  # 4 macro steps per batch
N_CC = T // P  # 16 natural-layout feature chunks per batch


def build_nc():
    import concourse.tile as tile
    from concourse import bacc, mybir

    f32 = mybir.dt.float32
    f32r = mybir.dt.float32r
    bf16 = mybir.dt.bfloat16
    AF = mybir.ActivationFunctionType

    nc = bacc.Bacc("TRN2", target_bir_lowering=False, debug=False, num_devices=N_CORES)

    fT_ext = nc.declare_dram_parameter("fT", [B_LOC, D, T], bf16, isOutput=False)
    fN_ext = nc.declare_dram_parameter("fN", [B_LOC, T, D], bf16, isOutput=False)
    hT_ext = nc.declare_dram_parameter("hT", [D, B_LOC], bf16, isOutput=False)
    w1_ext = nc.declare_dram_parameter("w1", [D, U], bf16, isOutput=False)
    w2_ext = nc.declare_dram_parameter("w2", [D, U], bf16, isOutput=False)
    v_ext = nc.declare_dram_parameter("vv", [U, 1], bf16, isOutput=False)
    b12_ext = nc.declare_dram_parameter("b12", [U], f32, isOutput=False)
    ctx_ext = nc.declare_dram_parameter("ctx_out", [B_LOC, D], f32, isOutput=True)
    attn_ext = nc.declare_dram_parameter("attn_out", [B_LOC, T], f32, isOutput=True)

    fT_ap = fT_ext.ap().rearrange("b (dc p) t -> p b dc t", p=P)  # [128, 4, 8, T]
    fN_ap = fN_ext.ap().rearrange("b (cc p) d -> p b cc d", p=P)  # [128, 4, 16, D]

    with tile.TileContext(nc) as tc:
        with (
            tc.tile_pool(name="consts", bufs=1) as consts,
            tc.tile_pool(name="ft_pool", bufs=2) as ft_pool,
            tc.tile_pool(name="fn_pool", bufs=24) as fn_pool,
            tc.tile_pool(name="s_pool", bufs=5) as s_pool,
            tc.tile_pool(name="row_pool", bufs=2) as row_pool,
            tc.tile_pool(name="small", bufs=4) as small,
            tc.tile_pool(name="proj_ps", bufs=2, space="PSUM") as proj_ps,
            tc.tile_pool(name="logit_ps", bufs=2, space="PSUM") as logit_ps,
            tc.tile_pool(name="misc_ps", bufs=2, space="PSUM") as misc_ps,
        ):
            # ---- constants / weights ----
            v_sb = consts.tile([P, UC], bf16)
            nc.sync.dma_start(
                out=v_sb, in_=v_ext.ap().rearrange("(uc p) o -> p (uc o)", p=P)
            )
            b12_sb = consts.tile([P, UC], f32)
            nc.sync.dma_start(
                out=b12_sb, in_=b12_ext.ap().rearrange("(uc p) -> p uc", p=P)
            )
            ones11_bf = consts.tile([1, 1], bf16)
            nc.vector.memset(ones11_bf, 1.0)

            # ---- weight/feature DMA order tuned for fast pipeline start:
            # hT + w2/w1 slice 0 + first feature tile first, then alternating
            # w2/w1 u-slices.  The ph matmul groups are interleaved into the
            # first macro-step's proj slots (emit_ph_group below) so the
            # in-order PE stream never waits on the full W2 load.
            ph_sb = consts.tile([P, UC, B_LOC], f32)
            w1_sb = consts.tile([P, DC, U], bf16)
            w2_sb = consts.tile([P, DC, U], bf16)
            hT_sb = consts.tile([P, DC, B_LOC], bf16)
            nc.sync.dma_start(
                out=hT_sb, in_=hT_ext.ap().rearrange("(dc p) b -> p dc b", p=P)
            )
            w1_ap = w1_ext.ap().rearrange("(dc p) u -> p dc u", p=P)
            w2_ap = w2_ext.ap().rearrange("(dc p) u -> p dc u", p=P)
            nc.sync.dma_start(out=w2_sb[:, :, 0:P], in_=w2_ap[:, :, 0:P])
            nc.sync.dma_start(out=w1_sb[:, :, 0:P], in_=w1_ap[:, :, 0:P])
            ft0 = ft_pool.tile([P, DC, TCH], bf16, tag="ft", name="ft0")
            nc.sync.dma_start(out=ft0, in_=fT_ap[:, 0, :, 0:TCH])
            for uc in range(1, UC):
                nc.sync.dma_start(
                    out=w2_sb[:, :, uc * P : (uc + 1) * P],
                    in_=w2_ap[:, :, uc * P : (uc + 1) * P],
                )
                nc.sync.dma_start(
                    out=w1_sb[:, :, uc * P : (uc + 1) * P],
                    in_=w1_ap[:, :, uc * P : (uc + 1) * P],
                )

            # ---- pipelined main loop over (b, tc) macro-steps ----
            l_sb = {}  # b -> [1, T] logits row (exp'd in place)
            e16_sb = {}  # b -> [1, T] bf16 copy of exp'd logits
            rinv = {}
            fn_tiles = {}  # (b, cc) -> bf16 [128, D]
            s_tiles = {}  # (tc, uc) -> score tile
            logit_tiles = {}  # tc -> psum tile [1, TCH]
            ft_cur = [None]

            def emit_ph_group(uc):
                ph_ps = misc_ps.tile([P, B_LOC], f32, tag="misc", name="ph_ps")
                for dc in range(DC):
                    nc.tensor.matmul(
                        ph_ps,
                        lhsT=w2_sb[:, dc, uc * P : (uc + 1) * P],
                        rhs=hT_sb[:, dc, :],
                        start=(dc == 0),
                        stop=(dc == DC - 1),
                    )
                nc.vector.tensor_scalar_add(
                    out=ph_sb[:, uc, :], in0=ph_ps, scalar1=b12_sb[:, uc : uc + 1]
                )

            def emit_proj(step, uc):
                b, tc = step
                ps = proj_ps.tile([P, TCH], f32, tag="proj")
                for dc in range(DC):
                    nc.tensor.matmul(
                        ps,
                        lhsT=w1_sb[:, dc, uc * P : (uc + 1) * P],
                        rhs=ft_cur[0][:, dc, :],
                        start=(dc == 0),
                        stop=(dc == DC - 1),
                    )
                s = s_pool.tile([P, TCH], bf16, tag="s")
                nc.scalar.activation(
                    out=s, in_=ps, func=AF.Tanh, bias=ph_sb[:, uc, b : b + 1], scale=1.0
                )
                s_tiles[(tc, uc)] = s

            def emit_v(step, uc):
                b, tc = step
                s = s_tiles.pop((tc, uc))
                if uc == 0:
                    logit_tiles[tc] = logit_ps.tile([1, TCH], f32, tag="lg", name="lg")
                nc.tensor.matmul(
                    logit_tiles[tc],
                    lhsT=v_sb[:, uc : uc + 1],
                    rhs=s,
                    start=(uc == 0),
                    stop=(uc == UC - 1),
                )
                if uc == UC - 1:
                    off = tc * TCH
                    nc.vector.tensor_copy(
                        out=l_sb[b][0:1, off : off + TCH], in_=logit_tiles.pop(tc)
                    )

            def emit_softmax(b):
                esum = small.tile([1, 1], f32, tag="es")
                # exp in place over the logits row, free running sum.
                # No max-subtraction: logits here are O(+-4) (tanh in [-1,1],
                # V ~ N(0, 1/U)); softmax is shift-invariant and fp32 exp is
                # safe far beyond that range.
                nc.scalar.activation(
                    out=l_sb[b], in_=l_sb[b], func=AF.Exp, bias=0.0, accum_out=esum
                )
                r = small.tile([1, 1], f32, tag="ri")
                nc.vector.reciprocal(out=r, in_=esum)
                rinv[b] = r
                e16 = row_pool.tile([1, T], bf16, tag="e16")
                nc.vector.tensor_copy(out=e16, in_=l_sb[b])
                e16_sb[b] = e16
                a = row_pool.tile([1, T], f32, tag="a")
                nc.vector.tensor_scalar_mul(out=a, in0=l_sb.pop(b), scalar1=r)
                nc.sync.dma_start(out=attn_ext.ap()[b : b + 1, :], in_=a)

            def emit_ctx(b):
                # transpose exp'd logits [1, T] into bf16 columns [128, T/128];
                # the softmax normalization is folded into the ctx copy scale.
                aT = small.tile([P, N_CC], bf16, tag="aT")
                for i in range(N_CC):
                    tp = misc_ps.tile([P, 1], f32, tag="misc")
                    nc.tensor.matmul(
                        tp,
                        lhsT=e16_sb[b][0:1, i * P : (i + 1) * P],
                        rhs=ones11_bf,
                        start=True,
                        stop=True,
                    )
                    nc.vector.tensor_copy(out=aT[:, i : i + 1], in_=tp)
                ctx_row = small.tile([1, D], f32, tag="ctx")
                for dh in range(D // TCH):
                    cps = misc_ps.tile([1, TCH], f32, tag="misc")
                    for cc in range(N_CC):
                        nc.tensor.matmul(
                            cps,
                            lhsT=aT[:, cc : cc + 1],
                            rhs=fn_tiles[(b, cc)][:, dh * TCH : (dh + 1) * TCH],
                            start=(cc == 0),
                            stop=(cc == N_CC - 1),
                        )
                    nc.vector.tensor_scalar_mul(
                        out=ctx_row[0:1, dh * TCH : (dh + 1) * TCH],
                        in0=cps,
                        scalar1=rinv[b],
                    )
                nc.sync.dma_start(out=ctx_ext.ap()[b : b + 1, :], in_=ctx_row)
                for cc in range(N_CC):
                    fn_tiles.pop((b, cc))
                e16_sb.pop(b)
                rinv.pop(b)

            steps = [(b, tc) for b in range(B_LOC) for tc in range(N_TC)]
            pending = []  # [(step, uc)] whose V matmuls are pending (2-slot lag)
            for step in steps:
                b, tc = step
                if tc == 0:
                    l_sb[b] = row_pool.tile([1, T], f32, tag="l", name="lrow")
                if b == 0 and tc == 0:
                    ft = ft0
                else:
                    ft = ft_pool.tile([P, DC, TCH], bf16, tag="ft")
                    nc.sync.dma_start(
                        out=ft, in_=fT_ap[:, b, :, tc * TCH : (tc + 1) * TCH]
                    )
                # natural-layout bf16 feature chunks for next batch's ctx phase
                for cc in range(tc * N_CC // N_TC, (tc + 1) * N_CC // N_TC):
                    fn = fn_pool.tile([P, D], bf16, tag="fn", name="fn")
                    nc.sync.dma_start(out=fn, in_=fN_ap[:, b, cc, :])
                    fn_tiles[(b, cc)] = fn

                for uc in range(UC):
                    ft_cur[0] = ft
                    if b == 0 and tc == 0:
                        emit_ph_group(uc)
                    emit_proj(step, uc)
                    pending.append((step, uc))
                    if len(pending) > 2:
                        pstep, puc = pending.pop(0)
                        emit_v(pstep, puc)
                        if puc == UC - 1 and pstep[1] == N_TC - 1:
                            emit_softmax(pstep[0])
                    if uc == 4 and tc == 0 and b >= 1:
                        emit_ctx(b - 1)

            for pstep, puc in pending:
                emit_v(pstep, puc)
            emit_softmax(B_LOC - 1)
            emit_ctx(B_LOC - 1)

    nc.compile()
    return nc


_NC_CACHE = None


def _get_nc():
    global _NC_CACHE
    if _NC_CACHE is None:
        _NC_CACHE = build_nc()
    return _NC_CACHE


def make_in_maps(features, hidden, W1, b1, W2, b2, V, bV):
    import ml_dtypes

    features = np.asarray(features, dtype=np.float32)
    hidden = np.asarray(hidden, dtype=np.float32)
    W1 = np.ascontiguousarray(np.asarray(W1, dtype=np.float32))
    W2 = np.ascontiguousarray(np.asarray(W2, dtype=np.float32))
    V = np.ascontiguousarray(np.asarray(V, dtype=np.float32).reshape(U, 1))
    b12 = np.asarray(b1, dtype=np.float32) + np.asarray(b2, dtype=np.float32)
    # bV is dropped: softmax output is invariant to a constant logit shift.

    in_maps = []
    for c in range(N_CORES):
        sl = slice(c * B_LOC, (c + 1) * B_LOC)
        f = features[sl]
        in_maps.append(
            {
                "fT": np.ascontiguousarray(f.transpose(0, 2, 1).astype(ml_dtypes.bfloat16)),
                "fN": np.ascontiguousarray(f.astype(ml_dtypes.bfloat16)),
                "hT": np.ascontiguousarray(hidden[sl].T.astype(ml_dtypes.bfloat16)),
                "w1": W1.astype(ml_dtypes.bfloat16),
                "w2": W2.astype(ml_dtypes.bfloat16),
                "vv": V.astype(ml_dtypes.bfloat16),
                "b12": b12,
            }
        )
    return in_maps


def kernel(features, hidden, W1, b1, W2, b2, V, bV):
    from concourse.bass_utils import run_bass_kernel_spmd

    nc = _get_nc()
    in_maps = make_in_maps(features, hidden, W1, b1, W2, b2, V, bV)
    res = run_bass_kernel_spmd(nc, in_maps, core_ids=list(range(N_CORES)))
    ctx = np.concatenate([res.results[c]["ctx_out"] for c in range(N_CORES)], axis=0)
    attn = np.concatenate([res.results[c]["attn_out"] for c in range(N_CORES)], axis=0)
    return ctx, attn[:, :, None]


# revision 15
# speedup vs baseline: 1.1916x; 1.0072x over previous
"""Bahdanau additive attention on 8 TRN2 NeuronCores (Bass/Tile).

Reference computation (per batch b):
    proj_f = features @ W1 + b1                         [T, U]
    proj_h = hidden @ W2 + b2                           [U]
    score  = tanh(proj_f + proj_h)                      [T, U]
    logits = score @ V + bV                             [T, 1]
    attn   = softmax(logits, axis=T)                    [T, 1]
    ctx    = sum_t attn[t] * features[t, :]             [D]
    returns (ctx [B, D], attn [B, T, 1])

Sharding: data-parallel over batch. B=32 across 8 cores -> 4 batches
per core; W1/W2/V replicated. No collectives.

Per-core layout:
  - The big matmul produces proj^T tiles [u=128, t=512]: W1 chunks
    [d=128, u=128] are the stationary operand, features^T [d=128,
    t=512] the moving operand (features fed pre-transposed per batch
    as fT [D, T]).  This makes proj_h a per-partition bias (fused into
    the tanh ACTIVATE, together with b1+b2) and makes the V
    contraction a partition-dim matmul.
  - Heavy matmuls run as float32r (full PE rate at N>=256), fp32 PSUM.
  - logits live as a [1, T] row per batch; softmax runs on one
    partition with fused exp+sum (ACT accum_out).  bV is dropped:
    softmax is shift-invariant, so bV cannot affect either output.
  - Attention weights are transposed to [t=128, 1] columns with tiny
    PE matmuls against [[1.0]] so the context matvec contracts t on
    the partition dim, streaming a second, natural-layout bf16 copy
    of features (fN).  bf16 halves both the DMA and the SBUF
    residency of that copy; context error stays ~1e-3.
  - Software pipelining: batch b's context phase is emitted in the
    middle of batch b+1's proj stream so the PE never waits on the
    softmax chain.
"""

import sys

sys.path.insert(0, "/opt/trn_rl_repo")

import numpy as np

B, T, D, U = 32, 2048, 1024, 1024
N_CORES = 8
B_LOC = B // N_CORES
P = 128
DC = D // P  # 8 d-chunks
UC = U // P  # 8 u-chunks
TCH = 512  # t-chunk (= one PSUM bank, = fp32 moving-operand max)
N_TC = T // TCH  # 4 macro steps per batch
N_CC = T // P  # 16 natural-layout feature chunks per batch


def build_nc():
    import concourse.tile as tile
    from concourse import bacc, mybir

    f32 = mybir.dt.float32
    f32r = mybir.dt.float32r
    bf16 = mybir.dt.bfloat16
    AF = mybir.ActivationFunctionType

    nc = bacc.Bacc("TRN2", target_bir_lowering=False, debug=False, num_devices=N_CORES)

    fT_ext = nc.declare_dram_parameter("fT", [B_LOC, D, T], bf16, isOutput=False)
    fN_ext = nc.declare_dram_parameter("fN", [B_LOC, T, D], bf16, isOutput=False)
    hT_ext = nc.declare_dram_parameter("hT", [D, B_LOC], bf16, isOutput=False)
    w1_ext = nc.declare_dram_parameter("w1", [D, U], bf16, isOutput=False)
    w2_ext = nc.declare_dram_parameter("w2", [D, U], bf16, isOutput=False)
    v_ext = nc.declare_dram_parameter("vv", [U, 1], bf16, isOutput=False)
    b12_ext = nc.declare_dram_parameter("b12", [U], f32, isOutput=False)
    ctx_ext = nc.declare_dram_parameter("ctx_out", [B_LOC, D], f32, isOutput=True)
    attn_ext = nc.declare_dram_parameter("attn_out", [B_LOC, T], f32, isOutput=True)

    fT_ap = fT_ext.ap().rearrange("b (dc p) t -> p b dc t", p=P)  # [128, 4, 8, T]
    fN_ap = fN_ext.ap().rearrange("b (cc p) d -> p b cc d", p=P)  # [128, 4, 16, D]

    with tile.TileContext(nc) as tc:
        with (
            tc.tile_pool(name="consts", bufs=1) as consts,
            tc.tile_pool(name="ft_pool", bufs=3) as ft_pool,
            tc.tile_pool(name="fn_pool", bufs=24) as fn_pool,
            tc.tile_pool(name="s_pool", bufs=5) as s_pool,
            tc.tile_pool(name="row_pool", bufs=2) as row_pool,
            tc.tile_pool(name="small", bufs=4) as small,
            tc.tile_pool(name="proj_ps", bufs=3, space="PSUM") as proj_ps,
            tc.tile_pool(name="logit_ps", bufs=2, space="PSUM") as logit_ps,
            tc.tile_pool(name="misc_ps", bufs=2, space="PSUM") as misc_ps,
        ):
            # ---- constants / weights ----
            v_sb = consts.tile([P, UC], bf16)
            nc.sync.dma_start(
                out=v_sb, in_=v_ext.ap().rearrange("(uc p) o -> p (uc o)", p=P)
            )
            b12_sb = consts.tile([P, UC], f32)
            nc.sync.dma_start(
                out=b12_sb, in_=b12_ext.ap().rearrange("(uc p) -> p uc", p=P)
            )
            ones11_bf = consts.tile([1, 1], bf16)
            nc.vector.memset(ones11_bf, 1.0)

            # ---- weight/feature DMA order tuned for fast pipeline start:
            # hT + w2/w1 slice 0 + first feature tile first, then alternating
            # w2/w1 u-slices.  The ph matmul groups are interleaved into the
            # first macro-step's proj slots (emit_ph_group below) so the
            # in-order PE stream never waits on the full W2 load.
            ph_sb = consts.tile([P, UC, B_LOC], f32)
            w1_sb = consts.tile([P, DC, U], bf16)
            w2_sb = consts.tile([P, DC, U], bf16)
            hT_sb = consts.tile([P, DC, B_LOC], bf16)
            nc.sync.dma_start(
                out=hT_sb, in_=hT_ext.ap().rearrange("(dc p) b -> p dc b", p=P)
            )
            w1_ap = w1_ext.ap().rearrange("(dc p) u -> p dc u", p=P)
            w2_ap = w2_ext.ap().rearrange("(dc p) u -> p dc u", p=P)
            nc.sync.dma_start(out=w2_sb[:, :, 0:P], in_=w2_ap[:, :, 0:P])
            nc.sync.dma_start(out=w1_sb[:, :, 0:P], in_=w1_ap[:, :, 0:P])
            ft0 = ft_pool.tile([P, DC, TCH], bf16, tag="ft", name="ft0")
            nc.sync.dma_start(out=ft0, in_=fT_ap[:, 0, :, 0:TCH])
            for uc in range(1, UC):
                nc.sync.dma_start(
                    out=w2_sb[:, :, uc * P : (uc + 1) * P],
                    in_=w2_ap[:, :, uc * P : (uc + 1) * P],
                )
                nc.sync.dma_start(
                    out=w1_sb[:, :, uc * P : (uc + 1) * P],
                    in_=w1_ap[:, :, uc * P : (uc + 1) * P],
                )

            # ---- pipelined main loop over (b, tc) macro-steps ----
            l_sb = {}  # b -> [1, T] logits row (exp'd in place)
            e16_sb = {}  # b -> [1, T] bf16 copy of exp'd logits
            rinv = {}
            fn_tiles = {}  # (b, cc) -> bf16 [128, D]
            s_tiles = {}  # (tc, uc) -> score tile
            logit_tiles = {}  # tc -> psum tile [1, TCH]
            ft_cur = [None]

            def emit_ph_group(uc):
                ph_ps = misc_ps.tile([P, B_LOC], f32, tag="misc", name="ph_ps")
                for dc in range(DC):
                    nc.tensor.matmul(
                        ph_ps,
                        lhsT=w2_sb[:, dc, uc * P : (uc + 1) * P],
                        rhs=hT_sb[:, dc, :],
                        start=(dc == 0),
                        stop=(dc == DC - 1),
                    )
                nc.vector.tensor_scalar_add(
                    out=ph_sb[:, uc, :], in0=ph_ps, scalar1=b12_sb[:, uc : uc + 1]
                )

            def emit_proj(step, uc):
                b, tc = step
                ps = proj_ps.tile([P, TCH], f32, tag="proj")
                for dc in range(DC):
                    nc.tensor.matmul(
                        ps,
                        lhsT=w1_sb[:, dc, uc * P : (uc + 1) * P],
                        rhs=ft_cur[0][:, dc, :],
                        start=(dc == 0),
                        stop=(dc == DC - 1),
                    )
                s = s_pool.tile([P, TCH], bf16, tag="s")
                nc.scalar.activation(
                    out=s, in_=ps, func=AF.Tanh, bias=ph_sb[:, uc, b : b + 1], scale=1.0
                )
                s_tiles[(tc, uc)] = s

            def emit_v(step, uc):
                b, tc = step
                s = s_tiles.pop((tc, uc))
                if uc == 0:
                    logit_tiles[tc] = logit_ps.tile([1, TCH], f32, tag="lg", name="lg")
                nc.tensor.matmul(
                    logit_tiles[tc],
                    lhsT=v_sb[:, uc : uc + 1],
                    rhs=s,
                    start=(uc == 0),
                    stop=(uc == UC - 1),
                )
                if uc == UC - 1:
                    off = tc * TCH
                    nc.vector.tensor_copy(
                        out=l_sb[b][0:1, off : off + TCH], in_=logit_tiles.pop(tc)
                    )

            def emit_softmax(b):
                esum = small.tile([1, 1], f32, tag="es")
                # exp in place over the logits row, free running sum.
                # No max-subtraction: logits here are O(+-4) (tanh in [-1,1],
                # V ~ N(0, 1/U)); softmax is shift-invariant and fp32 exp is
                # safe far beyond that range.
                nc.scalar.activation(
                    out=l_sb[b], in_=l_sb[b], func=AF.Exp, bias=0.0, accum_out=esum
                )
                r = small.tile([1, 1], f32, tag="ri")
                nc.vector.reciprocal(out=r, in_=esum)
                rinv[b] = r
                e16 = row_pool.tile([1, T], bf16, tag="e16")
                nc.vector.tensor_copy(out=e16, in_=l_sb[b])
                e16_sb[b] = e16
                a = row_pool.tile([1, T], f32, tag="a")
                nc.vector.tensor_scalar_mul(out=a, in0=l_sb.pop(b), scalar1=r)
                nc.sync.dma_start(out=attn_ext.ap()[b : b + 1, :], in_=a)

            def emit_ctx(b):
                # transpose exp'd logits [1, T] into bf16 columns [128, T/128];
                # the softmax normalization is folded into the ctx copy scale.
                aT = small.tile([P, N_CC], bf16, tag="aT")
                for i in range(N_CC):
                    tp = misc_ps.tile([P, 1], f32, tag="misc")
                    nc.tensor.matmul(
                        tp,
                        lhsT=e16_sb[b][0:1, i * P : (i + 1) * P],
                        rhs=ones11_bf,
                        start=True,
                        stop=True,
                    )
                    nc.vector.tensor_copy(out=aT[:, i : i + 1], in_=tp)
                ctx_row = small.tile([1, D], f32, tag="ctx")
                for dh in range(D // TCH):
                    cps = misc_ps.tile([1, TCH], f32, tag="misc")
                    for cc in range(N_CC):
                        nc.tensor.matmul(
                            cps,
                            lhsT=aT[:, cc : cc + 1],
                            rhs=fn_tiles[(b, cc)][:, dh * TCH : (dh + 1) * TCH],
                            start=(cc == 0),
                            stop=(cc == N_CC - 1),
                        )
                    nc.vector.tensor_scalar_mul(
                        out=ctx_row[0:1, dh * TCH : (dh + 1) * TCH],
                        in0=cps,
                        scalar1=rinv[b],
                    )
                nc.sync.dma_start(out=ctx_ext.ap()[b : b + 1, :], in_=ctx_row)
                for cc in range(N_CC):
                    fn_tiles.pop((b, cc))
                e16_sb.pop(b)
                rinv.pop(b)

            steps = [(b, tc) for b in range(B_LOC) for tc in range(N_TC)]
            pending = []  # [(step, uc)] whose V matmuls are pending (2-slot lag)
            for step in steps:
                b, tc = step
                if tc == 0:
                    l_sb[b] = row_pool.tile([1, T], f32, tag="l", name="lrow")
                if b == 0 and tc == 0:
                    ft = ft0
                else:
                    ft = ft_pool.tile([P, DC, TCH], bf16, tag="ft")
                    nc.sync.dma_start(
                        out=ft, in_=fT_ap[:, b, :, tc * TCH : (tc + 1) * TCH]
                    )
                # natural-layout bf16 feature chunks for next batch's ctx phase
                for cc in range(tc * N_CC // N_TC, (tc + 1) * N_CC // N_TC):
                    fn = fn_pool.tile([P, D], bf16, tag="fn", name="fn")
                    nc.sync.dma_start(out=fn, in_=fN_ap[:, b, cc, :])
                    fn_tiles[(b, cc)] = fn

                for uc in range(UC):
                    ft_cur[0] = ft
                    if b == 0 and tc == 0:
                        emit_ph_group(uc)
                    emit_proj(step, uc)
                    pending.append((step, uc))
                    if len(pending) > 2:
                        pstep, puc = pending.pop(0)
                        emit_v(pstep, puc)
                        if puc == UC - 1 and pstep[1] == N_TC - 1:
                            emit_softmax(pstep[0])
                    if uc == 4 and tc == 0 and b >= 1:
                        emit_ctx(b - 1)

            for pstep, puc in pending:
                emit_v(pstep, puc)
            emit_softmax(B_LOC - 1)
            emit_ctx(B_LOC - 1)

    nc.compile()
    return nc


_NC_CACHE = None


def _get_nc():
    global _NC_CACHE
    if _NC_CACHE is None:
        _NC_CACHE = build_nc()
    return _NC_CACHE


def make_in_maps(features, hidden, W1, b1, W2, b2, V, bV):
    import ml_dtypes

    features = np.asarray(features, dtype=np.float32)
    hidden = np.asarray(hidden, dtype=np.float32)
    W1 = np.ascontiguousarray(np.asarray(W1, dtype=np.float32))
    W2 = np.ascontiguousarray(np.asarray(W2, dtype=np.float32))
    V = np.ascontiguousarray(np.asarray(V, dtype=np.float32).reshape(U, 1))
    b12 = np.asarray(b1, dtype=np.float32) + np.asarray(b2, dtype=np.float32)
    # bV is dropped: softmax output is invariant to a constant logit shift.

    in_maps = []
    for c in range(N_CORES):
        sl = slice(c * B_LOC, (c + 1) * B_LOC)
        f = features[sl]
        in_maps.append(
            {
                "fT": np.ascontiguousarray(f.transpose(0, 2, 1).astype(ml_dtypes.bfloat16)),
                "fN": np.ascontiguousarray(f.astype(ml_dtypes.bfloat16)),
                "hT": np.ascontiguousarray(hidden[sl].T.astype(ml_dtypes.bfloat16)),
                "w1": W1.astype(ml_dtypes.bfloat16),
                "w2": W2.astype(ml_dtypes.bfloat16),
                "vv": V.astype(ml_dtypes.bfloat16),
                "b12": b12,
            }
        )
    return in_maps


def kernel(features, hidden, W1, b1, W2, b2, V, bV):
    from concourse.bass_utils import run_bass_kernel_spmd

    nc = _get_nc()
    in_maps = make_in_maps(features, hidden, W1, b1, W2, b2, V, bV)
    res = run_bass_kernel_spmd(nc, in_maps, core_ids=list(range(N_CORES)))
    ctx = np.concatenate([res.results[c]["ctx_out"] for c in range(N_CORES)], axis=0)
    attn = np.concatenate([res.results[c]["attn_out"] for c in range(N_CORES)], axis=0)
    return ctx, attn[:, :, None]


# revision 17
# speedup vs baseline: 1.1967x; 1.0043x over previous
"""Bahdanau additive attention on 8 TRN2 NeuronCores (Bass/Tile).

Reference computation (per batch b):
    proj_f = features @ W1 + b1                         [T, U]
    proj_h = hidden @ W2 + b2                           [U]
    score  = tanh(proj_f + proj_h)                      [T, U]
    logits = score @ V + bV                             [T, 1]
    attn   = softmax(logits, axis=T)                    [T, 1]
    ctx    = sum_t attn[t] * features[t, :]             [D]
    returns (ctx [B, D], attn [B, T, 1])

Sharding: data-parallel over batch. B=32 across 8 cores -> 4 batches
per core; W1/W2/V replicated. No collectives.

Per-core kernel structure:
  - The big matmul produces proj^T tiles [u=128, t=512]: W1 chunks
    [d=128, u=128] stationary, features^T [d=128, t=512] moving
    (features fed pre-transposed per batch as fT [D, T], bf16).  This
    orientation makes proj_h a per-partition bias (fused into the tanh
    ACTIVATE together with b1+b2) and makes the V contraction a
    partition-dim matmul.
  - bf16 inputs for all heavy matmuls (error ~2e-3 overall), fp32 PSUM
    accumulation everywhere.
  - Softmax is computed incrementally per 512-wide t-chunk: exp reads
    the logit PSUM directly (shift-invariant softmax, logits are
    O(+-4), so no max subtraction), with the running sum from the
    ACTIVATE's accum_out.  bV is dropped: it cannot affect outputs.
  - Exp'd logits are transposed into [t=128, 1] bf16 columns with tiny
    PE matmuls against [[1.0]]; the context matvec accumulates
    per-chunk into a batch-level PSUM pair, streaming a second,
    natural-layout bf16 copy of features (fN).  The softmax
    normalization happens once at batch end, folded into the
    PSUM-drain scale.
  - Software pipelining: V matmuls lag proj by 2 slots; each t-chunk's
    ctx work is emitted one macro-step later; weight DMAs are sliced
    so the PE starts ~5us into the NEFF.
"""

import sys

sys.path.insert(0, "/opt/trn_rl_repo")

import numpy as np

B, T, D, U = 32, 2048, 1024, 1024
N_CORES = 8
B_LOC = B // N_CORES
P = 128
DC = D // P  # 8 d-chunks
UC = U // P  # 8 u-chunks
TCH = 512  # t-chunk = one fp32 PSUM bank = fp32 moving-operand max
N_TC = T // TCH  # 4 macro steps per batch
CPT = TCH // P  # 4 transpose columns per t-chunk


def build_nc():
    import concourse.tile as tile
    from concourse import bacc, mybir

    f32 = mybir.dt.float32
    bf16 = mybir.dt.bfloat16
    AF = mybir.ActivationFunctionType

    nc = bacc.Bacc("TRN2", target_bir_lowering=False, debug=False, num_devices=N_CORES)

    fT_ext = nc.declare_dram_parameter("fT", [B_LOC, D, T], bf16, isOutput=False)
    fN_ext = nc.declare_dram_parameter("fN", [B_LOC, T, D], bf16, isOutput=False)
    hT_ext = nc.declare_dram_parameter("hT", [D, B_LOC], bf16, isOutput=False)
    w1_ext = nc.declare_dram_parameter("w1", [D, U], bf16, isOutput=False)
    w2_ext = nc.declare_dram_parameter("w2", [D, U], bf16, isOutput=False)
    v_ext = nc.declare_dram_parameter("vv", [U, 1], bf16, isOutput=False)
    b12_ext = nc.declare_dram_parameter("b12", [U], f32, isOutput=False)
    ctx_ext = nc.declare_dram_parameter("ctx_out", [B_LOC, D], f32, isOutput=True)
    attn_ext = nc.declare_dram_parameter("attn_out", [B_LOC, T], f32, isOutput=True)

    fT_ap = fT_ext.ap().rearrange("b (dc p) t -> p b dc t", p=P)  # [128, 4, 8, T]
    fN_ap = fN_ext.ap().rearrange("b (cc p) d -> p b cc d", p=P)  # [128, 4, 16, D]

    with tile.TileContext(nc) as tc:
        with (
            tc.tile_pool(name="consts", bufs=1) as consts,
            tc.tile_pool(name="ft_pool", bufs=3) as ft_pool,
            tc.tile_pool(name="fn_pool", bufs=10) as fn_pool,
            tc.tile_pool(name="s_pool", bufs=5) as s_pool,
            tc.tile_pool(name="row_pool", bufs=2) as row_pool,
            tc.tile_pool(name="small", bufs=4) as small,
            tc.tile_pool(name="proj_ps", bufs=2, space="PSUM") as proj_ps,
            tc.tile_pool(name="logit_ps", bufs=2, space="PSUM") as logit_ps,
            tc.tile_pool(name="ctx_ps", bufs=1, space="PSUM") as ctx_ps,
            tc.tile_pool(name="misc_ps", bufs=2, space="PSUM") as misc_ps,
        ):
            # ---- small constants ----
            v_sb = consts.tile([P, UC], bf16)
            nc.sync.dma_start(
                out=v_sb, in_=v_ext.ap().rearrange("(uc p) o -> p (uc o)", p=P)
            )
            b12_sb = consts.tile([P, UC], f32)
            nc.sync.dma_start(
                out=b12_sb, in_=b12_ext.ap().rearrange("(uc p) -> p uc", p=P)
            )
            ones11_bf = consts.tile([1, 1], bf16)
            nc.vector.memset(ones11_bf, 1.0)

            # ---- weight/feature DMAs ordered for a fast pipeline start ----
            ph_sb = consts.tile([P, UC, B_LOC], f32)
            w1_sb = consts.tile([P, DC, U], bf16)
            w2_sb = consts.tile([P, DC, U], bf16)
            hT_sb = consts.tile([P, DC, B_LOC], bf16)
            nc.sync.dma_start(
                out=hT_sb, in_=hT_ext.ap().rearrange("(dc p) b -> p dc b", p=P)
            )
            w1_ap = w1_ext.ap().rearrange("(dc p) u -> p dc u", p=P)
            w2_ap = w2_ext.ap().rearrange("(dc p) u -> p dc u", p=P)
            nc.sync.dma_start(out=w2_sb[:, :, 0:P], in_=w2_ap[:, :, 0:P])
            nc.sync.dma_start(out=w1_sb[:, :, 0:P], in_=w1_ap[:, :, 0:P])
            ft0 = ft_pool.tile([P, DC, TCH], bf16, tag="ft", name="ft0")
            nc.sync.dma_start(out=ft0, in_=fT_ap[:, 0, :, 0:TCH])
            for uc in range(1, UC):
                nc.sync.dma_start(
                    out=w2_sb[:, :, uc * P : (uc + 1) * P],
                    in_=w2_ap[:, :, uc * P : (uc + 1) * P],
                )
                nc.sync.dma_start(
                    out=w1_sb[:, :, uc * P : (uc + 1) * P],
                    in_=w1_ap[:, :, uc * P : (uc + 1) * P],
                )

            # ---- per-batch state ----
            e_row = {}  # b -> [1, T] f32 exp'd logits
            e16_row = {}  # b -> [1, T] bf16 exp'd logits
            esum4 = {}  # b -> [1, N_TC] f32 per-chunk exp sums
            aT16 = {}  # b -> [128, T/128] bf16 transposed exp'd logits
            ctx_pair = {}  # b -> (psum [1, TCH], psum [1, TCH])
            fn_tiles = {}  # (b, cc) -> bf16 [128, D]
            s_tiles = {}  # (tc, uc) -> bf16 score tile
            logit_tiles = {}  # tc -> psum [1, TCH]
            ft_cur = [None]

            def emit_ph_group(uc):
                ph_ps = misc_ps.tile([P, B_LOC], f32, tag="misc", name="ph_ps")
                for dc in range(DC):
                    nc.tensor.matmul(
                        ph_ps,
                        lhsT=w2_sb[:, dc, uc * P : (uc + 1) * P],
                        rhs=hT_sb[:, dc, :],
                        start=(dc == 0),
                        stop=(dc == DC - 1),
                    )
                nc.vector.tensor_scalar_add(
                    out=ph_sb[:, uc, :], in0=ph_ps, scalar1=b12_sb[:, uc : uc + 1]
                )

            def emit_proj(step, uc):
                b, tc = step
                ps = proj_ps.tile([P, TCH], f32, tag="proj")
                for dc in range(DC):
                    nc.tensor.matmul(
                        ps,
                        lhsT=w1_sb[:, dc, uc * P : (uc + 1) * P],
                        rhs=ft_cur[0][:, dc, :],
                        start=(dc == 0),
                        stop=(dc == DC - 1),
                    )
                s = s_pool.tile([P, TCH], bf16, tag="s")
                nc.scalar.activation(
                    out=s, in_=ps, func=AF.Tanh, bias=ph_sb[:, uc, b : b + 1], scale=1.0
                )
                s_tiles[(tc, uc)] = s

            def emit_v(step, uc):
                b, tc = step
                s = s_tiles.pop((tc, uc))
                if uc == 0:
                    logit_tiles[tc] = logit_ps.tile([1, TCH], f32, tag="lg", name="lg")
                nc.tensor.matmul(
                    logit_tiles[tc],
                    lhsT=v_sb[:, uc : uc + 1],
                    rhs=s,
                    start=(uc == 0),
                    stop=(uc == UC - 1),
                )
                if uc == UC - 1:
                    # exp straight out of the logit PSUM (no max subtraction:
                    # logits are O(+-4) here and softmax is shift-invariant),
                    # running sum via accum_out.
                    off = tc * TCH
                    lg = logit_tiles.pop(tc)
                    nc.scalar.activation(
                        out=e_row[b][0:1, off : off + TCH],
                        in_=lg,
                        func=AF.Exp,
                        bias=0.0,
                        accum_out=esum4[b][0:1, tc : tc + 1],
                    )
                    nc.vector.tensor_copy(
                        out=e16_row[b][0:1, off : off + TCH],
                        in_=e_row[b][0:1, off : off + TCH],
                    )

            def emit_finish_batch(b):
                esum = small.tile([1, 1], f32, tag="es")
                nc.vector.tensor_reduce(
                    out=esum,
                    in_=esum4[b],
                    axis=mybir.AxisListType.X,
                    op=mybir.AluOpType.add,
                )
                r = small.tile([1, 1], f32, tag="ri")
                nc.vector.reciprocal(out=r, in_=esum)
                a = row_pool.tile([1, T], f32, tag="a")
                nc.vector.tensor_scalar_mul(out=a, in0=e_row.pop(b), scalar1=r)
                nc.sync.dma_start(out=attn_ext.ap()[b : b + 1, :], in_=a)
                ctx_row = small.tile([1, D], f32, tag="ctx")
                for dh in range(D // TCH):
                    nc.vector.tensor_scalar_mul(
                        out=ctx_row[0:1, dh * TCH : (dh + 1) * TCH],
                        in0=ctx_pair[b][dh],
                        scalar1=r,
                    )
                nc.sync.dma_start(out=ctx_ext.ap()[b : b + 1, :], in_=ctx_row)
                ctx_pair.pop(b)
                e16_row.pop(b)
                aT16.pop(b)
                esum4.pop(b)
                for cc in range(T // P):
                    fn_tiles.pop((b, cc))

            def emit_ctx_part(b, tc):
                # transpose this chunk's exp'd logits into bf16 columns and
                # accumulate the chunk's contribution to the ctx matvec.
                if tc == 0:
                    aT16[b] = small.tile([P, T // P], bf16, tag="aT", name="aT16")
                    ctx_pair[b] = (
                        ctx_ps.tile([1, TCH], f32, tag="ctx0", name="cps0"),
                        ctx_ps.tile([1, TCH], f32, tag="ctx1", name="cps1"),
                    )
                for i in range(CPT):
                    cc = tc * CPT + i
                    tp = misc_ps.tile([P, 1], f32, tag="misc", name="tp")
                    nc.tensor.matmul(
                        tp,
                        lhsT=e16_row[b][0:1, cc * P : (cc + 1) * P],
                        rhs=ones11_bf,
                        start=True,
                        stop=True,
                    )
                    nc.vector.tensor_copy(out=aT16[b][:, cc : cc + 1], in_=tp)
                for dh in range(D // TCH):
                    for i in range(CPT):
                        cc = tc * CPT + i
                        nc.tensor.matmul(
                            ctx_pair[b][dh],
                            lhsT=aT16[b][:, cc : cc + 1],
                            rhs=fn_tiles[(b, cc)][:, dh * TCH : (dh + 1) * TCH],
                            start=(cc == 0),
                            stop=(cc == T // P - 1),
                        )
                if tc == N_TC - 1:
                    emit_finish_batch(b)

            steps = [(b, tc) for b in range(B_LOC) for tc in range(N_TC)]
            pending = []  # [(step, uc)] V matmuls pending (2-slot lag)
            for m, step in enumerate(steps):
                b, tc = step
                if tc == 0:
                    e_row[b] = row_pool.tile([1, T], f32, tag="e", name="erow")
                    e16_row[b] = row_pool.tile([1, T], bf16, tag="e16", name="e16row")
                    esum4[b] = small.tile([1, N_TC], f32, tag="es4", name="esum4")
                if b == 0 and tc == 0:
                    ft = ft0
                else:
                    ft = ft_pool.tile([P, DC, TCH], bf16, tag="ft")
                    nc.sync.dma_start(
                        out=ft, in_=fT_ap[:, b, :, tc * TCH : (tc + 1) * TCH]
                    )
                for i in range(CPT):
                    cc = tc * CPT + i
                    fn = fn_pool.tile([P, D], bf16, tag="fn", name="fn")
                    nc.sync.dma_start(out=fn, in_=fN_ap[:, b, cc, :])
                    fn_tiles[(b, cc)] = fn

                for uc in range(UC):
                    ft_cur[0] = ft
                    if b == 0 and tc == 0:
                        emit_ph_group(uc)
                    emit_proj(step, uc)
                    pending.append((step, uc))
                    if len(pending) > 2:
                        emit_v(*pending.pop(0))
                    if uc == 3 and m >= 1:
                        pb, ptc = steps[m - 1]
                        emit_ctx_part(pb, ptc)

            for pstep, puc in pending:
                emit_v(pstep, puc)
            emit_ctx_part(B_LOC - 1, N_TC - 1)

    nc.compile()
    return nc


_NC_CACHE = None


def _get_nc():
    global _NC_CACHE
    if _NC_CACHE is None:
        _NC_CACHE = build_nc()
    return _NC_CACHE


def make_in_maps(features, hidden, W1, b1, W2, b2, V, bV):
    import ml_dtypes

    features = np.asarray(features, dtype=np.float32)
    hidden = np.asarray(hidden, dtype=np.float32)
    W1 = np.ascontiguousarray(np.asarray(W1, dtype=np.float32))
    W2 = np.ascontiguousarray(np.asarray(W2, dtype=np.float32))
    V = np.ascontiguousarray(np.asarray(V, dtype=np.float32).reshape(U, 1))
    b12 = np.asarray(b1, dtype=np.float32) + np.asarray(b2, dtype=np.float32)
    # bV is dropped: softmax output is invariant to a constant logit shift.

    in_maps = []
    for c in range(N_CORES):
        sl = slice(c * B_LOC, (c + 1) * B_LOC)
        f = features[sl]
        in_maps.append(
            {
                "fT": np.ascontiguousarray(
                    f.transpose(0, 2, 1).astype(ml_dtypes.bfloat16)
                ),
                "fN": np.ascontiguousarray(f.astype(ml_dtypes.bfloat16)),
                "hT": np.ascontiguousarray(hidden[sl].T.astype(ml_dtypes.bfloat16)),
                "w1": W1.astype(ml_dtypes.bfloat16),
                "w2": W2.astype(ml_dtypes.bfloat16),
                "vv": V.astype(ml_dtypes.bfloat16),
                "b12": b12,
            }
        )
    return in_maps


def kernel(features, hidden, W1, b1, W2, b2, V, bV):
    from concourse.bass_utils import run_bass_kernel_spmd

    nc = _get_nc()
    in_maps = make_in_maps(features, hidden, W1, b1, W2, b2, V, bV)
    res = run_bass_kernel_spmd(nc, in_maps, core_ids=list(range(N_CORES)))
    ctx = np.concatenate([res.results[c]["ctx_out"] for c in range(N_CORES)], axis=0)
    attn = np.concatenate([res.results[c]["attn_out"] for c in range(N_CORES)], axis=0)
    return ctx, attn[:, :, None]
